# revision 14
# baseline (speedup 1.0000x reference)
"""Trainium2 Bass kernel for nn_MANO1D (galerkin linear attention, 8 cores).

Algebraic collapse: with no nonlinearity between the projections, the whole
module reduces to  out[b] = queries[b] @ G[b] + bout  with

    Sraw[b] = keys[b]^T @ values[b]                      # [64, 64]
    G[b]    = sum_h U_h @ Sraw[b] @ Z_h                  # [64, 64]
    U_h     = Wq_h^T @ Wk_h                              # host precomputed
    Z_h     = (Wout_h @ Wk_h)^T / N                      # host precomputed

Sharding: core c handles (batch b = c//2, half = c%2 of the sequence).
Each core computes a partial Sraw over its 8192 rows of keys/values, the
(linear-in-S) partial G, then the partial output  queries[b] @ G_partial
over the full sequence.  The host sums the two partials per batch and adds
bout.  No cross-core communication needed.

Device layouts (everything 128-partition for full DMA bandwidth):
  kv  [128, 8192]  chunk-major: kv[p, 128c+e] = concat(K,V)[128c+p, e]
  qt  [128, 8192]  rows 0:64 = Q^T[:, :8192], rows 64:128 = Q^T[:, 8192:]
  wz  [64, 512]    Z packed: col block j = Z_{2j} (j<4), col 256+64j = Z_{2j+1}
  wu  [128, 256]   U^T pairs: [:, 64j:] = vstack(U_{2j}^T, U_{2j+1}^T)
  ot  [128, 8192]  output, same packing as qt

v2 critical-path fixes over v1 (measured on the v1 trace):
  - kv split 50:50 across the two HWDGE rings (they are symmetric, ~220
    GB/s each) and chunked 2x per ring, so Sraw closes ~4.8us after the
    first packet instead of 10.5us.
  - G chain shortened: Sraw^T duplicated onto both partition halves by
    vector+scalar in parallel, Y computed as two concurrent quadrant
    matmuls into one [128,256] psum tile, G as 4 accumulating matmuls
    with 128-deep contraction (was 8 with 64-deep), casts in parallel.
  - weights in bf16 (halves w DMA), everything bf16 end to end.
"""

import ml_dtypes
import numpy as np

import concourse.bacc as bacc
import concourse.mybir as mybir
import concourse.tile as tile
from concourse.bass_utils import run_bass_kernel_spmd

B, N, D, H = 4, 16384, 64, 8
HALF = N // 2            # 8192 rows of k/v per core; qt/ot free size
CH = HALF // 128         # 64 contraction chunks for Sraw
NT = HALF // 512         # 16 output column tiles per half
KVG = 4                  # kv transfer groups (2 per ring, 16 chunks each)

_cached = None


def _build():
    global _cached
    if _cached is not None:
        return _cached

    f32 = mybir.dt.float32
    bf16 = mybir.dt.bfloat16

    nc = bacc.Bacc("TRN2", debug=False, num_devices=8, enable_asserts=False)
    # Drop the constructor preamble we don't use: the four const-AP memsets
    # (nothing reads them here) and the entry all-engine butterfly (~2.9 us on
    # HW).  Body ordering is fully covered by Tile-generated semaphores, and
    # NRT zero-initializes semaphores at load.
    _entry = nc.m.functions[0].blocks[0]
    _entry.instructions[:] = [
        i
        for i in _entry.instructions
        if not (
            str(getattr(i, "opcode", "")).endswith(("Memset", "Drain"))
            or str(i.name).startswith("barrier_")
        )
    ]
    kv_ap = nc.dram_tensor("kv", [128, CH * 128], bf16, kind="ExternalInput").ap()
    qt_ap = nc.dram_tensor("qt", [128, HALF], bf16, kind="ExternalInput").ap()
    wz_ap = nc.dram_tensor("wz", [64, 512], bf16, kind="ExternalInput").ap()
    wu_ap = nc.dram_tensor("wu", [128, 256], bf16, kind="ExternalInput").ap()
    ot_ap = nc.dram_tensor("ot", [128, HALF], bf16, kind="ExternalOutput").ap()

    with tile.TileContext(nc) as tc:
        with (
            tc.tile_pool(name="data", bufs=1) as data,
            tc.tile_pool(name="small", bufs=1) as small,
            tc.tile_pool(name="ps", bufs=1, space="PSUM") as ps,
            tc.tile_pool(name="psout", bufs=6, space="PSUM") as psout,
        ):
            kv_sb = data.tile([128, CH * 128], bf16)
            qt_sb = data.tile([128, HALF], bf16)
            ot_sb = data.tile([128, HALF], bf16)
            wz_sb = small.tile([64, 512], bf16)
            wu_sb = small.tile([128, 256], bf16)
            st_sb = small.tile([64, 64], bf16)
            g_sb = small.tile([128, 64], bf16)

            # kv absolutely first, 50:50 across BOTH HWDGE rings, two
            # transfers each (4KB/partition runs keep the descriptor
            # generator ahead of the wire).  Then weights, then qt in
            # 2048-col groups alternating rings.
            kvq = CH * 128 // KVG  # 2048 cols per kv transfer group
            for g in range(KVG):
                ring = nc.sync if g % 2 == 0 else nc.scalar
                ring.dma_start(kv_sb[:, g * kvq : (g + 1) * kvq],
                               kv_ap[:, g * kvq : (g + 1) * kvq])
            nc.scalar.dma_start(wz_sb[:], wz_ap[:])
            nc.scalar.dma_start(wu_sb[:], wu_ap[:])
            qt_grp = 2048
            for g in range(HALF // qt_grp):
                c0 = g * qt_grp
                ring = nc.sync if g % 2 == 0 else nc.scalar
                ring.dma_start(qt_sb[:, c0 : c0 + qt_grp], qt_ap[:, c0 : c0 + qt_grp])

            # Phase 1: Sraw^T = V^T K over 64 chunks of 128 rows.  Even/odd
            # chunks accumulate into disjoint psum partition halves (col
            # tiling) so consecutive matmuls overlap on the PE.  Chunk c
            # only depends on kv transfer group c//16, so the matmuls trail
            # the kv stream.
            ps_st = ps.tile([128, 64], f32, tag="sm", bufs=2)
            for c in range(CH):
                p0 = 64 * (c % 2)
                nc.tensor.matmul(
                    ps_st[p0 : p0 + 64, :],
                    lhsT=kv_sb[:, c * 128 + 64 : c * 128 + 128],
                    rhs=kv_sb[:, c * 128 : c * 128 + 64],
                    start=(c < 2),
                    stop=(c >= CH - 2),
                )
            # Sraw^T (copy then in-place add: DVE reads at most one PSUM
            # operand per op)
            nc.vector.tensor_copy(st_sb[:], ps_st[0:64, :])
            nc.vector.tensor_add(st_sb[:], st_sb[:], ps_st[64:128, :])

            # Y packed as [128, 256]: rows 0:64 = [Y_0|Y_2|Y_4|Y_6],
            # rows 64:128 = [Y_1|Y_3|Y_5|Y_7] (out psum base is independent
            # of the input partitions, so no duplicated st needed).
            ps_y = ps.tile([128, 256], f32, tag="sm", bufs=2)
            nc.tensor.matmul(ps_y[0:64, :], lhsT=st_sb[:],
                             rhs=wz_sb[:, 0:256], start=True, stop=True)
            nc.tensor.matmul(ps_y[64:128, :], lhsT=st_sb[:],
                             rhs=wz_sb[:, 256:512], start=True, stop=True)
            y_sb = small.tile([128, 256], bf16)
            nc.vector.tensor_copy(y_sb[:, 0:128], ps_y[:, 0:128])
            nc.scalar.copy(y_sb[:, 128:256], ps_y[:, 128:256])

            # G = sum_j [U_2j^T; U_2j+1^T]^T @ [Y_2j; Y_2j+1]: 4 accumulating
            # matmuls with full 128-deep contraction.
            ps_g = ps.tile([64, 64], f32, tag="sm", bufs=2)
            for j in range(4):
                nc.tensor.matmul(
                    ps_g[:],
                    lhsT=wu_sb[:, 64 * j : 64 * j + 64],
                    rhs=y_sb[:, 64 * j : 64 * j + 64],
                    start=(j == 0),
                    stop=(j == 3),
                )
            nc.vector.tensor_copy(g_sb[0:64, :], ps_g[:])
            nc.scalar.copy(g_sb[64:128, :], ps_g[:])

            # Phase 2: out^T = G^T @ Q^T.  The two sequence halves sit on
            # partition ranges 0:64 / 64:128: their matmuls land in disjoint
            # (row, col) quadrants of the PE array and disjoint partition
            # halves of ONE psum bank, so they run concurrently and a single
            # [128, 512] copy drains both.
            for t in range(NT):
                c0 = t * 512
                po = psout.tile([128, 512], f32)
                for half in (0, 1):
                    p0 = 64 * half
                    nc.tensor.matmul(
                        po[p0 : p0 + 64, :],
                        lhsT=g_sb[p0 : p0 + 64, :],
                        rhs=qt_sb[p0 : p0 + 64, c0 : c0 + 512],
                        start=True,
                        stop=True,
                    )
                copy = nc.vector.tensor_copy if t % 2 == 0 else nc.scalar.copy
                copy(ot_sb[:, c0 : c0 + 512], po[:])

            # Store in 2048-col groups alternating rings (4KB/partition
            # runs), short 1024-col tail so the last store is quick.
            for c0, c1, ring in (
                (0, 2048, nc.sync),
                (2048, 4096, nc.scalar),
                (4096, 6144, nc.sync),
                (6144, 7168, nc.scalar),
                (7168, 8192, nc.sync),
            ):
                ring.dma_start(ot_ap[:, c0:c1], ot_sb[:, c0:c1])

    # Tail surgery: Tile's epilogue is [store-completion drain, barrier #1,
    # semaphore range-clear, barrier #2].  Barrier #2 only makes every engine
    # wait for the clear; NEFF completion already requires each engine stream
    # (clear included) to finish, so drop everything after the clear.
    for bb in nc.m.functions[0].blocks:
        if bb.name.endswith("_end"):
            insts = bb.instructions
            isa_idx = [
                i
                for i, x in enumerate(insts)
                if str(getattr(x, "opcode", "")).endswith("ISA")
            ]
            if isa_idx:
                del insts[isa_idx[-1] + 1 :]

    nc.compile()
    _cached = nc
    return nc


def kernel(queries, keys, values, Wq, Wk, Wout, bout):
    queries = np.asarray(queries, np.float32)
    keys = np.asarray(keys, np.float32)
    values = np.asarray(values, np.float32)
    Wq = np.asarray(Wq, np.float32)
    Wk = np.asarray(Wk, np.float32)
    Wout = np.asarray(Wout, np.float32)
    bout = np.asarray(bout, np.float32)

    nc = _build()

    # Host precompute of the folded weight matrices (tiny).
    wz = np.empty((64, 512), np.float32)
    wu = np.empty((128, 256), np.float32)
    for h in range(H):
        Wq_h = Wq[64 * h : 64 * h + 64, :]
        Wk_h = Wk[64 * h : 64 * h + 64, :]
        Wout_h = Wout[:, 64 * h : 64 * h + 64]
        U_hT = Wk_h.T @ Wq_h            # U_h^T,  U_h = Wq_h^T @ Wk_h
        Z_h = (Wout_h @ Wk_h).T / np.float32(N)
        j, r = divmod(h, 2)
        wz[:, 64 * (j + 4 * r) : 64 * (j + 4 * r) + 64] = Z_h
        wu[64 * r : 64 * r + 64, 64 * j : 64 * j + 64] = U_hT
    wz_in = np.ascontiguousarray(wz).astype(ml_dtypes.bfloat16)
    wu_in = np.ascontiguousarray(wu).astype(ml_dtypes.bfloat16)

    in_maps = []
    for c in range(8):
        b, half = c // 2, c % 2
        r0 = half * HALF
        kv_rows = np.concatenate(
            [keys[b, r0 : r0 + HALF], values[b, r0 : r0 + HALF]], axis=1
        )  # [8192, 128]
        kv = np.ascontiguousarray(
            kv_rows.reshape(CH, 128, 128).transpose(1, 0, 2).reshape(128, CH * 128)
        ).astype(ml_dtypes.bfloat16)
        qT = queries[b].T  # [64, 16384]
        qtp = np.ascontiguousarray(
            np.concatenate([qT[:, :HALF], qT[:, HALF:]], axis=0)
        ).astype(ml_dtypes.bfloat16)
        in_maps.append({"kv": kv, "qt": qtp, "wz": wz_in, "wu": wu_in})

    res = run_bass_kernel_spmd(nc, in_maps, core_ids=list(range(8)))

    out = np.empty((B, N, D), np.float32)
    for b in range(B):
        s = res.results[2 * b]["ot"].astype(np.float32) + res.results[2 * b + 1][
            "ot"
        ].astype(np.float32)  # [128, 8192]
        outT = np.concatenate([s[0:64], s[64:128]], axis=1)  # [64, 16384]
        out[b] = outT.T + bout
    return out


# revision 16
# speedup vs baseline: 1.0227x; 1.0227x over previous
"""Trainium2 Bass kernel for nn_MANO1D (galerkin linear attention, 8 cores).

Algebraic collapse: with no nonlinearity between the projections, the whole
module reduces to  out[b] = queries[b] @ G[b] + bout  with

    Sraw[b] = keys[b]^T @ values[b]                      # [64, 64]
    G[b]    = sum_h U_h @ Sraw[b] @ Z_h                  # [64, 64]
    U_h     = Wq_h^T @ Wk_h                              # host precomputed
    Z_h     = (Wout_h @ Wk_h)^T / N                      # host precomputed

Sharding: core c handles (batch b = c//2, half = c%2 of the sequence).
Each core computes a partial Sraw over its 8192 rows of keys/values, the
(linear-in-S) partial G, then the partial output  queries[b] @ G_partial
over the full sequence.  The host sums the two partials per batch and adds
bout.  No cross-core communication needed.

Device layouts (everything 128-partition for full DMA bandwidth):
  kv  [128, 8192]  chunk-major: kv[p, 128c+e] = concat(K,V)[128c+p, e]
  qt  [128, 8192]  rows 0:64 = Q^T[:, :8192], rows 64:128 = Q^T[:, 8192:]
  wz  [64, 512]    Z packed: col block j = Z_{2j} (j<4), col 256+64j = Z_{2j+1}
  wu  [128, 256]   U^T pairs: [:, 64j:] = vstack(U_{2j}^T, U_{2j+1}^T)
  ot  [128, 8192]  output, same packing as qt

v2 critical-path fixes over v1 (measured on the v1 trace):
  - kv split 50:50 across the two HWDGE rings (they are symmetric, ~220
    GB/s each) and chunked 2x per ring, so Sraw closes ~4.8us after the
    first packet instead of 10.5us.
  - G chain shortened: Sraw^T duplicated onto both partition halves by
    vector+scalar in parallel, Y computed as two concurrent quadrant
    matmuls into one [128,256] psum tile, G as 4 accumulating matmuls
    with 128-deep contraction (was 8 with 64-deep), casts in parallel.
  - weights in bf16 (halves w DMA), everything bf16 end to end.
"""

import ml_dtypes
import numpy as np

import concourse.bacc as bacc
import concourse.mybir as mybir
import concourse.tile as tile
from concourse.bass_utils import run_bass_kernel_spmd

B, N, D, H = 4, 16384, 64, 8
HALF = N // 2            # 8192 rows of k/v per core; qt/ot free size
CH = HALF // 128         # 64 contraction chunks for Sraw
NT = HALF // 512         # 16 output column tiles per half
KVG = 4                  # kv transfer groups (2 per ring, 16 chunks each)

_cached = None


def _build():
    global _cached
    if _cached is not None:
        return _cached

    f32 = mybir.dt.float32
    bf16 = mybir.dt.bfloat16

    nc = bacc.Bacc("TRN2", debug=False, num_devices=8, enable_asserts=False)
    # Drop the constructor preamble we don't use: the four const-AP memsets
    # (nothing reads them here) and the entry all-engine butterfly (~2.9 us on
    # HW).  Body ordering is fully covered by Tile-generated semaphores, and
    # NRT zero-initializes semaphores at load.
    _entry = nc.m.functions[0].blocks[0]
    _entry.instructions[:] = [
        i
        for i in _entry.instructions
        if not (
            str(getattr(i, "opcode", "")).endswith(("Memset", "Drain"))
            or str(i.name).startswith("barrier_")
        )
    ]
    kv_ap = nc.dram_tensor("kv", [128, CH * 128], bf16, kind="ExternalInput").ap()
    qt_ap = nc.dram_tensor("qt", [128, HALF], bf16, kind="ExternalInput").ap()
    wz_ap = nc.dram_tensor("wz", [64, 512], bf16, kind="ExternalInput").ap()
    wu_ap = nc.dram_tensor("wu", [128, 256], bf16, kind="ExternalInput").ap()
    ot_ap = nc.dram_tensor("ot", [128, HALF], bf16, kind="ExternalOutput").ap()

    with tile.TileContext(nc) as tc:
        with (
            tc.tile_pool(name="data", bufs=1) as data,
            tc.tile_pool(name="small", bufs=1) as small,
            tc.tile_pool(name="ps", bufs=1, space="PSUM") as ps,
            tc.tile_pool(name="psout", bufs=6, space="PSUM") as psout,
        ):
            kv_sb = data.tile([128, CH * 128], bf16)
            qt_sb = data.tile([128, HALF], bf16)
            ot_sb = data.tile([128, HALF], bf16)
            wz_sb = small.tile([64, 512], bf16)
            wu_sb = small.tile([128, 256], bf16)
            st_sb = small.tile([64, 64], bf16)
            g_sb = small.tile([128, 64], bf16)

            # kv absolutely first, 50:50 across BOTH HWDGE rings, two
            # transfers each (4KB/partition runs keep the descriptor
            # generator ahead of the wire).  Then weights, then qt in
            # 2048-col groups alternating rings.
            kvq = CH * 128 // KVG  # 2048 cols per kv transfer group
            for g in range(KVG):
                ring = nc.sync if g % 2 == 0 else nc.scalar
                ring.dma_start(kv_sb[:, g * kvq : (g + 1) * kvq],
                               kv_ap[:, g * kvq : (g + 1) * kvq])
            nc.scalar.dma_start(wz_sb[:], wz_ap[:])
            nc.scalar.dma_start(wu_sb[:], wu_ap[:])
            # The SDMA engines drain one HWDGE queue with strong hysteresis:
            # once qt descriptors sit in a queue they can starve the OTHER
            # queue's kv (measured: kv_g2 landed at 15.7us while qt streamed
            # from 10us).  Gate each qt dma_start on kv completion with a
            # fabricated WAR dependency: a [1,1] copy from the last kv group
            # into the qt group's SBUF region makes the dma_start (and thus
            # its descriptor injection) wait for kv data.
            qt_grp = 2048
            for g in range(HALF // qt_grp):
                c0 = g * qt_grp
                nc.vector.tensor_copy(
                    qt_sb[0:1, c0 : c0 + 1],
                    kv_sb[0:1, (KVG - 1) * kvq + g : (KVG - 1) * kvq + g + 1],
                )
                ring = nc.sync if g % 2 == 0 else nc.scalar
                ring.dma_start(qt_sb[:, c0 : c0 + qt_grp], qt_ap[:, c0 : c0 + qt_grp])

            # Phase 1: Sraw^T = V^T K over 64 chunks of 128 rows.  Even/odd
            # chunks accumulate into disjoint psum partition halves (col
            # tiling) so consecutive matmuls overlap on the PE.  Chunk c
            # only depends on kv transfer group c//16, so the matmuls trail
            # the kv stream.
            ps_st = ps.tile([128, 64], f32, tag="sm", bufs=2)
            for c in range(CH):
                p0 = 64 * (c % 2)
                nc.tensor.matmul(
                    ps_st[p0 : p0 + 64, :],
                    lhsT=kv_sb[:, c * 128 + 64 : c * 128 + 128],
                    rhs=kv_sb[:, c * 128 : c * 128 + 64],
                    start=(c < 2),
                    stop=(c >= CH - 2),
                )
            # Sraw^T (copy then in-place add: DVE reads at most one PSUM
            # operand per op)
            nc.vector.tensor_copy(st_sb[:], ps_st[0:64, :])
            nc.vector.tensor_add(st_sb[:], st_sb[:], ps_st[64:128, :])

            # Y packed as [128, 256]: rows 0:64 = [Y_0|Y_2|Y_4|Y_6],
            # rows 64:128 = [Y_1|Y_3|Y_5|Y_7] (out psum base is independent
            # of the input partitions, so no duplicated st needed).
            ps_y = ps.tile([128, 256], f32, tag="sm", bufs=2)
            nc.tensor.matmul(ps_y[0:64, :], lhsT=st_sb[:],
                             rhs=wz_sb[:, 0:256], start=True, stop=True)
            nc.tensor.matmul(ps_y[64:128, :], lhsT=st_sb[:],
                             rhs=wz_sb[:, 256:512], start=True, stop=True)
            y_sb = small.tile([128, 256], bf16)
            nc.vector.tensor_copy(y_sb[:, 0:128], ps_y[:, 0:128])
            nc.scalar.copy(y_sb[:, 128:256], ps_y[:, 128:256])

            # G = sum_j [U_2j^T; U_2j+1^T]^T @ [Y_2j; Y_2j+1]: 4 accumulating
            # matmuls with full 128-deep contraction.
            ps_g = ps.tile([64, 64], f32, tag="sm", bufs=2)
            for j in range(4):
                nc.tensor.matmul(
                    ps_g[:],
                    lhsT=wu_sb[:, 64 * j : 64 * j + 64],
                    rhs=y_sb[:, 64 * j : 64 * j + 64],
                    start=(j == 0),
                    stop=(j == 3),
                )
            nc.vector.tensor_copy(g_sb[0:64, :], ps_g[:])
            nc.scalar.copy(g_sb[64:128, :], ps_g[:])

            # Phase 2: out^T = G^T @ Q^T.  The two sequence halves sit on
            # partition ranges 0:64 / 64:128: their matmuls land in disjoint
            # (row, col) quadrants of the PE array and disjoint partition
            # halves of ONE psum bank, so they run concurrently and a single
            # [128, 512] copy drains both.
            for t in range(NT):
                c0 = t * 512
                po = psout.tile([128, 512], f32)
                for half in (0, 1):
                    p0 = 64 * half
                    nc.tensor.matmul(
                        po[p0 : p0 + 64, :],
                        lhsT=g_sb[p0 : p0 + 64, :],
                        rhs=qt_sb[p0 : p0 + 64, c0 : c0 + 512],
                        start=True,
                        stop=True,
                    )
                copy = nc.vector.tensor_copy if t % 2 == 0 else nc.scalar.copy
                copy(ot_sb[:, c0 : c0 + 512], po[:])

            # Stores: small leading group so the stream starts after two
            # copies, 2048-col middles, small tail split across rings.
            for c0, c1, ring in (
                (0, 1024, nc.sync),
                (1024, 3072, nc.scalar),
                (3072, 5120, nc.sync),
                (5120, 7168, nc.scalar),
                (7168, 8192, nc.sync),
            ):
                ring.dma_start(ot_ap[:, c0:c1], ot_sb[:, c0:c1])

    # Tail surgery: Tile's epilogue is [store-completion drain, barrier #1,
    # semaphore range-clear, barrier #2].  Barrier #2 only makes every engine
    # wait for the clear; NEFF completion already requires each engine stream
    # (clear included) to finish, so drop everything after the clear.
    for bb in nc.m.functions[0].blocks:
        if bb.name.endswith("_end"):
            insts = bb.instructions
            isa_idx = [
                i
                for i, x in enumerate(insts)
                if str(getattr(x, "opcode", "")).endswith("ISA")
            ]
            if isa_idx:
                del insts[isa_idx[-1] + 1 :]

    nc.compile()
    _cached = nc
    return nc


def kernel(queries, keys, values, Wq, Wk, Wout, bout):
    queries = np.asarray(queries, np.float32)
    keys = np.asarray(keys, np.float32)
    values = np.asarray(values, np.float32)
    Wq = np.asarray(Wq, np.float32)
    Wk = np.asarray(Wk, np.float32)
    Wout = np.asarray(Wout, np.float32)
    bout = np.asarray(bout, np.float32)

    nc = _build()

    # Host precompute of the folded weight matrices (tiny).
    wz = np.empty((64, 512), np.float32)
    wu = np.empty((128, 256), np.float32)
    for h in range(H):
        Wq_h = Wq[64 * h : 64 * h + 64, :]
        Wk_h = Wk[64 * h : 64 * h + 64, :]
        Wout_h = Wout[:, 64 * h : 64 * h + 64]
        U_hT = Wk_h.T @ Wq_h            # U_h^T,  U_h = Wq_h^T @ Wk_h
        Z_h = (Wout_h @ Wk_h).T / np.float32(N)
        j, r = divmod(h, 2)
        wz[:, 64 * (j + 4 * r) : 64 * (j + 4 * r) + 64] = Z_h
        wu[64 * r : 64 * r + 64, 64 * j : 64 * j + 64] = U_hT
    wz_in = np.ascontiguousarray(wz).astype(ml_dtypes.bfloat16)
    wu_in = np.ascontiguousarray(wu).astype(ml_dtypes.bfloat16)

    in_maps = []
    for c in range(8):
        b, half = c // 2, c % 2
        r0 = half * HALF
        kv_rows = np.concatenate(
            [keys[b, r0 : r0 + HALF], values[b, r0 : r0 + HALF]], axis=1
        )  # [8192, 128]
        kv = np.ascontiguousarray(
            kv_rows.reshape(CH, 128, 128).transpose(1, 0, 2).reshape(128, CH * 128)
        ).astype(ml_dtypes.bfloat16)
        qT = queries[b].T  # [64, 16384]
        qtp = np.ascontiguousarray(
            np.concatenate([qT[:, :HALF], qT[:, HALF:]], axis=0)
        ).astype(ml_dtypes.bfloat16)
        in_maps.append({"kv": kv, "qt": qtp, "wz": wz_in, "wu": wu_in})

    res = run_bass_kernel_spmd(nc, in_maps, core_ids=list(range(8)))

    out = np.empty((B, N, D), np.float32)
    for b in range(B):
        s = res.results[2 * b]["ot"].astype(np.float32) + res.results[2 * b + 1][
            "ot"
        ].astype(np.float32)  # [128, 8192]
        outT = np.concatenate([s[0:64], s[64:128]], axis=1)  # [64, 16384]
        out[b] = outT.T + bout
    return out
